# revision 1
# baseline (speedup 1.0000x reference)
"""Multi-head attention (RoPE + pos_bias + mask) Trainium2 Bass kernel.

Sharding: tensor-parallel over heads (2 heads per core, 8 cores), both
batch elements on every core.  Each core computes its heads' attention
and a partial o_proj (its slice of the contraction dim); the host sums
the 8 partials and adds b_o.

All matmuls run as float32r (~12-bit mantissa, full fp32 range).
pos_bias and the mask are combined host-side into one additive bf16
bias in logits-transposed layout; masked entries get -30000 so exp
underflows to exactly 0 (reference uses -9e15 + rowmax subtraction,
identical post-softmax).  Softmax runs without max subtraction (logits
are O(5)); denominators come from a ones-column appended to V.
"""
import numpy as np
import ml_dtypes

import concourse.bass as bass
import concourse.mybir as mybir
import concourse.tile as tile
from concourse.bass_utils import run_bass_kernel_spmd

B, S, D, H, HD = 2, 2048, 1024, 16, 64
NCORES = 8
T = B * S            # 4096 tokens
KO = D // 128        # 8 contraction subtiles
MASK_NEG = -30000.0

F32 = mybir.dt.float32
F32R = mybir.dt.float32r
BF16 = mybir.dt.bfloat16
AF = mybir.ActivationFunctionType

TRACE = False
LAST_RESULT = None   # BassKernelResults of the most recent run (for profiling)

_waitfix_ctr = [0]


def _split_waits(nc, max_waits=1):
    """walrus in this environment accepts only one sync-wait command per
    instruction; TileContext emits several on some (notably the tail
    drain).  Move extras onto single-wait NoOps inserted just before, on
    the same engine queue — identical ordering semantics."""
    total = 0
    for fn in nc.m.functions:
        for bb in fn.blocks:
            out = []
            changed = False
            for ins in bb.instructions:
                si = ins.sync_info
                if si is not None and si.on_wait and len(si.on_wait) > max_waits:
                    waits = list(si.on_wait)
                    for w in waits[:-max_waits]:
                        _waitfix_ctr[0] += 1
                        n = mybir.InstNoOp(
                            name=f"I-waitfix-{_waitfix_ctr[0]}",
                            ins=[], outs=[], engine=ins.engine,
                        )
                        n.sync_info = mybir.SyncInfo(on_wait=[w], on_update=[])
                        out.append(n)
                        total += 1
                    ins.sync_info = mybir.SyncInfo(
                        on_wait=waits[-max_waits:],
                        on_update=list(si.on_update or []),
                    )
                    changed = True
                out.append(ins)
            if changed:
                bb.instructions = out
    return total


def _build():
    nc = bass.Bass()
    xT = nc.declare_dram_parameter("xT", [128, KO, T], F32R, isOutput=False)
    wqk = nc.declare_dram_parameter("wqk", [128, KO, 256], F32R, isOutput=False)
    wqkb = nc.declare_dram_parameter("wqkb", [2, 256], F32R, isOutput=False)
    wv = nc.declare_dram_parameter("wv", [128, KO, 128], F32R, isOutput=False)
    wvb = nc.declare_dram_parameter("wvb", [2, 128], F32R, isOutput=False)
    wo = nc.declare_dram_parameter("wo", [128, D], F32R, isOutput=False)
    cos2 = nc.declare_dram_parameter("cos2", [128, T], F32, isOutput=False)
    sinsh = nc.declare_dram_parameter("sinsh", [128, T], F32, isOutput=False)
    biasd = nc.declare_dram_parameter("bias", [2, 16, 128, S], BF16,
                                      isOutput=False)
    outp = nc.declare_dram_parameter("out", [T, D], F32, isOutput=True)

    with tile.TileContext(nc) as tc:
        with (
            tc.tile_pool(name="const", bufs=1) as cst,
            tc.tile_pool(name="persist", bufs=1) as pers,
        ):
            wqk_sb = cst.tile([128, KO, 256], F32R)
            nc.sync.dma_start(wqk_sb[:], wqk[:])
            wv_sb = cst.tile([128, KO, 128], F32R)
            nc.sync.dma_start(wv_sb[:], wv[:])
            wo_sb = cst.tile([128, D], F32R)
            nc.sync.dma_start(wo_sb[:], wo[:])
            wqkb_sb = cst.tile([2, 256], F32R)
            nc.sync.dma_start(wqkb_sb[:], wqkb[:])
            wvb_sb = cst.tile([2, 128], F32R)
            nc.sync.dma_start(wvb_sb[:], wvb[:])
            ones2 = cst.tile([2, 512], F32R)
            nc.vector.memset(ones2[:].bitcast(F32), 0.0)
            nc.vector.memset(ones2[0:1, :].bitcast(F32), 1.0)
            ones2x64 = cst.tile([2, 64], F32R)
            nc.vector.memset(ones2x64[:].bitcast(F32), 0.0)
            nc.vector.memset(ones2x64[0:1, :].bitcast(F32), 1.0)

            qT = pers.tile([128, T], F32R)
            kT = pers.tile([128, T], F32R)
            v1 = pers.tile([128, 32, 130], F32R)
            valsT = pers.tile([128, T], F32R)
            nc.vector.memset(v1[:, :, 64:65].bitcast(F32), 1.0)
            nc.vector.memset(v1[:, :, 129:130].bitcast(F32), 1.0)

            # ---------------- Phase A: qkv projection + rope ----------------
            with (
                tc.tile_pool(name="trig", bufs=1) as trig,
                tc.tile_pool(name="pa", bufs=3) as pa,
                tc.tile_pool(name="pap", bufs=2, space="PSUM") as pap,
            ):
                cos_sb = trig.tile([128, T], F32)
                nc.sync.dma_start(cos_sb[:], cos2[:])
                sin_sb = trig.tile([128, T], F32)
                nc.sync.dma_start(sin_sb[:], sinsh[:])
                for ch in range(T // 512):
                    cs = ch * 512
                    xc = pa.tile([128, KO, 512], F32R, tag="xc")
                    for ko in range(KO):   # per-ko DMAs ride separate queues
                        nc.sync.dma_start(xc[:, ko], xT[:, ko, cs:cs + 512])
                    for m in range(2):          # 0 = q, 1 = k
                        pq = pap.tile([128, 512], F32, tag="pq")
                        for ko in range(KO):
                            nc.tensor.matmul(
                                pq[:], wqk_sb[:, ko, m * 128:(m + 1) * 128],
                                xc[:, ko], start=(ko == 0), stop=False)
                        nc.tensor.matmul(
                            pq[:], wqkb_sb[:, m * 128:(m + 1) * 128],
                            ones2[:], start=False, stop=True)
                        t1 = pa.tile([128, 512], F32, tag="t1")
                        rot = pa.tile([128, 512], F32, tag="rot")
                        nc.vector.tensor_mul(
                            out=t1[:], in0=pq[:], in1=cos_sb[:, cs:cs + 512])
                        for hl in range(2):
                            b0 = 64 * hl
                            nc.vector.tensor_mul(
                                out=rot[b0:b0 + 32, :],
                                in0=pq[b0 + 32:b0 + 64, :],
                                in1=sin_sb[b0:b0 + 32, cs:cs + 512])
                            nc.vector.tensor_mul(
                                out=rot[b0 + 32:b0 + 64, :],
                                in0=pq[b0:b0 + 32, :],
                                in1=sin_sb[b0 + 32:b0 + 64, cs:cs + 512])
                        dst = qT if m == 0 else kT
                        nc.vector.tensor_add(
                            out=dst[:, cs:cs + 512], in0=t1[:], in1=rot[:])
                    for tt in range(4):         # v in [token, dim] layout
                        g = ch * 4 + tt
                        pv = pap.tile([128, 128], F32, tag="pvv")
                        for ko in range(KO):
                            nc.tensor.matmul(
                                pv[:], xc[:, ko, tt * 128:(tt + 1) * 128],
                                wv_sb[:, ko], start=(ko == 0), stop=False)
                        nc.tensor.matmul(
                            pv[:], ones2[:, 0:128], wvb_sb[:],
                            start=False, stop=True)
                        nc.vector.tensor_copy(out=v1[:, g, 0:64],
                                              in_=pv[:, 0:64])
                        nc.vector.tensor_copy(out=v1[:, g, 65:129],
                                              in_=pv[:, 64:128])

            # ---------------- Phase B: attention ----------------
            # Inner loop interleaves both heads (disjoint PE row groups so
            # LDWEIGHTS overlaps the other head's matmul) and skews the PV
            # matmuls 2 kt-iterations behind the logits matmuls so the PE
            # FIFO never blocks on the DVE-add -> ACT-exp chain.
            SKEW = 4   # in (hl, kt) steps; 4 = 2 full kt iterations
            with (
                tc.tile_pool(name="pb", bufs=6) as pb,
                tc.tile_pool(name="pbias", bufs=14) as pbias,
                tc.tile_pool(name="pbn", bufs=2) as pbn,
                tc.tile_pool(name="pbp", bufs=3, space="PSUM") as pbp,
                tc.tile_pool(name="pvp", bufs=4, space="PSUM") as pvp,
                tc.tile_pool(name="bcp", bufs=1, space="PSUM") as bcp,
            ):
                for b in range(2):
                    for qc in range(4):
                        qs = qc * 512
                        qtok = b * S + qs
                        pvt = [pvp.tile([65, 512], F32, tag="pv",
                                        name=f"pv_{b}_{qc}_{hl}")
                               for hl in range(2)]
                        pend = []
                        for kt in range(16):
                            ktok = b * S + kt * 128
                            # burst both heads' logits back-to-back so the
                            # PE stream is L,L then PV,PV (same-shape runs
                            # keep LDWEIGHTS overlapped)
                            for hl in range(2):
                                h0 = 64 * hl
                                bias_sb = pbias.tile([128, 512], BF16,
                                                     tag="bias")
                                nc.sync.dma_start(
                                    bias_sb[:],
                                    biasd[hl, kt, :, qs:qs + 512])
                                pl = pbp.tile([128, 512], F32, tag="pl")
                                nc.tensor.matmul(
                                    pl[:],
                                    kT[h0:h0 + 64, ktok:ktok + 128],
                                    qT[h0:h0 + 64, qtok:qtok + 512],
                                    start=True, stop=True)
                                nc.vector.tensor_add(
                                    out=pl[:], in0=pl[:], in1=bias_sb[:])
                                ex = pb.tile([128, 512], F32R, tag="ex")
                                nc.scalar.activation(ex[:], pl[:], AF.Exp)
                                pend.append((hl, kt, ex))
                            while len(pend) > SKEW:
                                fhl, fkt, fex = pend.pop(0)
                                nc.tensor.matmul(
                                    pvt[fhl][:],
                                    v1[:, b * 16 + fkt,
                                       65 * fhl:65 * fhl + 65],
                                    fex[:],
                                    start=(fkt == 0), stop=(fkt == 15),
                                    skip_group_check=True)
                        for fhl, fkt, fex in pend:
                            nc.tensor.matmul(
                                pvt[fhl][:],
                                v1[:, b * 16 + fkt, 65 * fhl:65 * fhl + 65],
                                fex[:],
                                start=(fkt == 0), stop=(fkt == 15),
                                skip_group_check=True)
                        for hl in range(2):
                            h0 = 64 * hl
                            rec = pbn.tile([1, 512], F32, tag="rec")
                            nc.vector.reciprocal(rec[:], pvt[hl][64:65, :])
                            rec2 = pbn.tile([2, 512], F32R, tag="rec2")
                            # row 1 must be finite: 0-weight x NaN = NaN
                            nc.vector.memset(rec2[:].bitcast(F32), 0.0)
                            nc.vector.tensor_copy(out=rec2[0:1, :],
                                                  in_=rec[:])
                            bc = bcp.tile([64, 512], F32, tag="bc")
                            nc.tensor.matmul(bc[:], ones2x64[:], rec2[:],
                                             start=True, stop=True)
                            bcs = pbn.tile([64, 512], F32, tag="bcs")
                            nc.scalar.copy(bcs[:], bc[:])
                            nc.vector.tensor_mul(
                                out=valsT[h0:h0 + 64, qtok:qtok + 512],
                                in0=pvt[hl][0:64, :], in1=bcs[:])

            # ---------------- Phase C: partial o_proj ----------------
            with (
                tc.tile_pool(name="pc", bufs=4) as pc,
                tc.tile_pool(name="pcp", bufs=3, space="PSUM") as pcp,
            ):
                for mt in range(T // 128):
                    for n2 in range(2):
                        po = pcp.tile([128, 512], F32, tag="po")
                        nc.tensor.matmul(
                            po[:], valsT[:, mt * 128:(mt + 1) * 128],
                            wo_sb[:, n2 * 512:(n2 + 1) * 512],
                            start=True, stop=True)
                        ob = pc.tile([128, 512], F32, tag="ob",
                                     name=f"ob_{mt}_{n2}")
                        nc.any.tensor_copy(out=ob[:], in_=po[:])
                        nc.sync.dma_start(
                            outp[mt * 128:(mt + 1) * 128,
                                 n2 * 512:(n2 + 1) * 512], ob[:])

    _split_waits(nc)
    return nc


_nc_cache = None


def _get_nc():
    global _nc_cache
    if _nc_cache is None:
        _nc_cache = _build()
    return _nc_cache


def _prep_inputs(x, pos_bias, sinusoidal_pos, mask, W_qkv, b_qkv, W_o, b_o):
    """Build the 8 per-core input maps (all host-side layout prep)."""
    x = np.asarray(x, np.float32)
    pos_bias = np.asarray(pos_bias, np.float32)
    sp = np.asarray(sinusoidal_pos, np.float32)[0, 0]        # [S, HD]
    mask = np.asarray(mask)
    W_qkv = np.asarray(W_qkv, np.float32)
    b_qkv = np.asarray(b_qkv, np.float32)
    W_o = np.asarray(W_o, np.float32)

    scale = np.float32(1.0 / np.sqrt(HD))

    xflat = x.reshape(T, D)
    xT_np = np.ascontiguousarray(
        xflat.T.reshape(KO, 128, T).transpose(1, 0, 2))       # [128, KO, T]

    cos_t = np.cos(sp).T.astype(np.float32)                   # [HD, S]
    sin_t = np.sin(sp).T.astype(np.float32)
    cos2_np = np.ascontiguousarray(np.tile(cos_t, (2, B)))    # [128, T]
    sinsh64 = np.concatenate([-sin_t[:HD // 2], sin_t[HD // 2:]], axis=0)
    sinsh_np = np.ascontiguousarray(np.tile(sinsh64, (2, B)))

    # additive mask term in logits-T layout [k, q]
    maskT = np.where(mask[0, 0].T == 0, np.float32(MASK_NEG),
                     np.float32(0.0)).astype(np.float32)      # [S(k), S(q)]

    # per-head W rows: feature f = h*192 + j (j<64 q, <128 k, <192 v)
    Wh = W_qkv.reshape(H, 3 * HD, D)
    bh = b_qkv.reshape(H, 3 * HD)

    in_maps = []
    for c in range(NCORES):
        h0, h1 = 2 * c, 2 * c + 1
        # q rows scaled by 1/sqrt(HD); k rows unscaled
        Wqk_c = np.concatenate([
            Wh[h0, 0:HD] * scale, Wh[h1, 0:HD] * scale,
            Wh[h0, HD:2 * HD], Wh[h1, HD:2 * HD]], axis=0)    # [256, D]
        bqk_c = np.concatenate([
            bh[h0, 0:HD] * scale, bh[h1, 0:HD] * scale,
            bh[h0, HD:2 * HD], bh[h1, HD:2 * HD]], axis=0)    # [256]
        Wv_c = np.concatenate([Wh[h0, 2 * HD:], Wh[h1, 2 * HD:]], axis=0)
        bv_c = np.concatenate([bh[h0, 2 * HD:], bh[h1, 2 * HD:]], axis=0)

        wqk_np = np.ascontiguousarray(
            Wqk_c.T.reshape(KO, 128, 256).transpose(1, 0, 2))  # [128, KO, 256]
        wv_np = np.ascontiguousarray(
            Wv_c.T.reshape(KO, 128, 128).transpose(1, 0, 2))
        wqkb_np = np.zeros((2, 256), np.float32)
        wqkb_np[0] = bqk_c
        wvb_np = np.zeros((2, 128), np.float32)
        wvb_np[0] = bv_c
        wo_np = np.ascontiguousarray(W_o[:, 128 * c:128 * (c + 1)].T)  # [128, D]

        bias_np = np.empty((2, 16, 128, S), ml_dtypes.bfloat16)
        for hl in range(2):
            bt = pos_bias[0, 2 * c + hl].T * scale + maskT     # [S(k), S(q)]
            bias_np[hl] = bt.reshape(16, 128, S).astype(ml_dtypes.bfloat16)

        in_maps.append({
            "xT": xT_np, "wqk": wqk_np, "wqkb": wqkb_np,
            "wv": wv_np, "wvb": wvb_np, "wo": wo_np,
            "cos2": cos2_np, "sinsh": sinsh_np, "bias": bias_np,
        })
    return in_maps


def _ensure_profile_hook():
    """Register the axon NTFF profiling hook if the image lacks
    antenv.axon_hooks (needed only for TRACE=True runs)."""
    import sys
    import types
    try:
        from antenv.axon_hooks import get_axon_ntff_profile_hook  # noqa
        return
    except ImportError:
        pass
    try:
        from trn_agent_boot.trn_boot import _ntff_profile_via_ctypes
        hook = _ntff_profile_via_ctypes("/opt/axon/libaxon_pjrt.so")
        mod = types.ModuleType("antenv.axon_hooks")
        mod.get_axon_ntff_profile_hook = lambda: hook
        mod.set_axon_ntff_profile_hook = lambda h: None
        sys.modules["antenv.axon_hooks"] = mod
    except Exception:
        pass


def kernel(x, pos_bias, sinusoidal_pos, mask, W_qkv, b_qkv, W_o, b_o):
    global LAST_RESULT
    if TRACE:
        _ensure_profile_hook()
    in_maps = _prep_inputs(x, pos_bias, sinusoidal_pos, mask,
                           W_qkv, b_qkv, W_o, b_o)
    nc = _get_nc()
    try:
        r = run_bass_kernel_spmd(nc, in_maps, list(range(NCORES)),
                                 trace=TRACE)
    except Exception:
        # occasional transient NRT device errors — retry once
        r = run_bass_kernel_spmd(nc, in_maps, list(range(NCORES)),
                                 trace=TRACE)
    LAST_RESULT = r
    acc = np.zeros((T, D), np.float64)
    for c in range(NCORES):
        acc += r.results[c]["out"].astype(np.float64)
    out = (acc + np.asarray(b_o, np.float32).astype(np.float64)).astype(
        np.float32)
    return out.reshape(B, S, D)



# revision 11
# speedup vs baseline: 1.7050x; 1.7050x over previous
"""Multi-head attention (RoPE + pos_bias + mask) Trainium2 Bass kernel.

Sharding: tensor-parallel over heads (2 heads per core, 8 cores), both
batch elements on every core.  Each core computes its heads' attention
and a partial o_proj (its slice of the contraction dim); the host sums
the 8 partials and adds b_o.

v2 design notes (vs the 613us baseline):
- All matmuls run in fp16 (1 cycle/row on the PE; f32r pays 4x below a
  256-wide moving dim, which made the v-projection 4x slow).
- The additive (pos_bias + mask) term is factored out of the softmax:
  ex = exp(qk) * expb with expb = exp(pos_bias*scale) * (mask != 0)
  precomputed host-side in fp16.  This removes the 658ns/tile fp32
  PSUM-source DVE adds (the old kernel's largest DVE cost) and replaces
  them with 2x-rate all-fp16 SBUF multiplies; masked entries become
  exact zeros so no -30000 logits are needed anywhere.
- Both heads' logits for one (kt, 512q) tile land in one [128,1024]
  PSUM tile (two adjacent banks), so each exp is a single N=1024 ACT
  instruction (997ns) instead of two N=512 (1440ns).
- Denominators ride as a ones-column inside the V stationary operand.
- DMA payloads are fp16 with >=2KB contiguous lines per partition.
"""
import numpy as np

import concourse.bass as bass
import concourse.mybir as mybir
import concourse.tile as tile
from concourse.bass_utils import run_bass_kernel_spmd

B, S, D, H, HD = 2, 2048, 1024, 16, 64
NCORES = 8
T = B * S            # 4096 tokens
KO = D // 128        # 8 contraction subtiles
NCH = T // 512       # 8 projection chunks
QC = S // 512        # 4 q-chunks per batch

F32 = mybir.dt.float32
F16 = mybir.dt.float16
AF = mybir.ActivationFunctionType

TRACE = False
LAST_RESULT = None   # BassKernelResults of the most recent run (for profiling)

_waitfix_ctr = [0]


def _split_waits(nc, max_waits=1):
    """walrus in this environment accepts only one sync-wait command per
    instruction; TileContext emits several on some (notably the tail
    drain).  Move extras onto single-wait NoOps inserted just before, on
    the same engine queue — identical ordering semantics."""
    total = 0
    for fn in nc.m.functions:
        for bb in fn.blocks:
            out = []
            changed = False
            for ins in bb.instructions:
                si = ins.sync_info
                if si is not None and si.on_wait and len(si.on_wait) > max_waits:
                    waits = list(si.on_wait)
                    for w in waits[:-max_waits]:
                        _waitfix_ctr[0] += 1
                        n = mybir.InstNoOp(
                            name=f"I-waitfix-{_waitfix_ctr[0]}",
                            ins=[], outs=[], engine=ins.engine,
                        )
                        n.sync_info = mybir.SyncInfo(on_wait=[w], on_update=[])
                        out.append(n)
                        total += 1
                    ins.sync_info = mybir.SyncInfo(
                        on_wait=waits[-max_waits:],
                        on_update=list(si.on_update or []),
                    )
                    changed = True
                out.append(ins)
            if changed:
                bb.instructions = out
    return total


def _build():
    nc = bass.Bass()
    xT = nc.declare_dram_parameter("xT", [128, NCH, KO, 512], F16,
                                   isOutput=False)
    wqk = nc.declare_dram_parameter("wqk", [128, KO, 256], F16, isOutput=False)
    wqkb = nc.declare_dram_parameter("wqkb", [2, 256], F16, isOutput=False)
    wv = nc.declare_dram_parameter("wv", [128, KO, 128], F16, isOutput=False)
    wvb = nc.declare_dram_parameter("wvb", [2, 128], F16, isOutput=False)
    wo = nc.declare_dram_parameter("wo", [128, D], F16, isOutput=False)
    cosd = nc.declare_dram_parameter("cosd", [64, S], F16, isOutput=False)
    sind = nc.declare_dram_parameter("sind", [64, S], F16, isOutput=False)
    # expb[qc, kp, kt, 0:512]   = exp(pos_bias)*mask for head0
    # expb[qc, kp, kt, 512:1024] = same for head1  (k = kt*128 + kp)
    expbd = nc.declare_dram_parameter("expb", [QC, 128, 16, 1024], F16,
                                      isOutput=False)
    outp = nc.declare_dram_parameter("out", [T, D], F16, isOutput=True)

    with tile.TileContext(nc) as tc:
        with (
            tc.tile_pool(name="const", bufs=1) as cst,
            tc.tile_pool(name="persist", bufs=1) as pers,
        ):
            wqk_sb = cst.tile([128, KO, 256], F16)
            nc.sync.dma_start(wqk_sb[:], wqk[:])
            wv_sb = cst.tile([128, KO, 128], F16)
            nc.sync.dma_start(wv_sb[:], wv[:])
            wo_sb = cst.tile([128, D], F16)
            nc.sync.dma_start(wo_sb[:], wo[:])
            wqkb_sb = cst.tile([2, 256], F16)
            nc.sync.dma_start(wqkb_sb[:], wqkb[:])
            wvb_sb = cst.tile([2, 128], F16)
            nc.sync.dma_start(wvb_sb[:], wvb[:])
            ones2 = cst.tile([2, 512], F16)
            nc.vector.memset(ones2[:], 0.0)
            nc.vector.memset(ones2[0:1, :], 1.0)
            ones2x64 = cst.tile([2, 64], F16)
            nc.vector.memset(ones2x64[:], 0.0)
            nc.vector.memset(ones2x64[0:1, :], 1.0)

            cos_sb = cst.tile([128, T], F16)
            sin_sb = cst.tile([128, T], F16)
            for hp in range(2):          # partition halves (head0 / head1)
                for bb_ in range(2):     # token halves (batch0 / batch1)
                    nc.sync.dma_start(
                        cos_sb[64 * hp:64 * hp + 64,
                               S * bb_:S * bb_ + S], cosd[:])
                    nc.sync.dma_start(
                        sin_sb[64 * hp:64 * hp + 64,
                               S * bb_:S * bb_ + S], sind[:])

            qT = pers.tile([128, T], F16)
            kT = pers.tile([128, T], F16)
            # v1[:, g, 0:64] = head0 dims, col 64 = ones, 65:129 = head1
            # dims, col 129 = ones  (g = token tile of 128)
            v1 = pers.tile([128, 32, 130], F16)
            valsT = pers.tile([128, T], F16)
            nc.vector.memset(v1[:, :, 64:65], 1.0)
            nc.vector.memset(v1[:, :, 129:130], 1.0)

            # ---------------- Phase A: qkv projection + rope ----------------
            with (
                tc.tile_pool(name="pa", bufs=2) as pa,
                tc.tile_pool(name="par", bufs=3) as par,
                tc.tile_pool(name="pap", bufs=2, space="PSUM") as pap,
                tc.tile_pool(name="pavp", bufs=2, space="PSUM") as pavp,
            ):
                for ch in range(NCH):
                    cs = ch * 512
                    xc = pa.tile([128, KO, 512], F16, tag="xc")
                    nc.sync.dma_start(xc[:], xT[:, ch])
                    for m in range(2):          # 0 = q, 1 = k
                        pq = pap.tile([128, 512], F32, tag="pq")
                        for ko in range(KO):
                            nc.tensor.matmul(
                                pq[:], wqk_sb[:, ko, m * 128:(m + 1) * 128],
                                xc[:, ko], start=(ko == 0), stop=False)
                        nc.tensor.matmul(
                            pq[:], wqkb_sb[:, m * 128:(m + 1) * 128],
                            ones2[:], start=False, stop=True)
                        pq16 = par.tile([128, 512], F16, tag="pq16")
                        nc.scalar.copy(pq16[:], pq[:])
                        t1 = par.tile([128, 512], F16, tag="t1")
                        rot = par.tile([128, 512], F16, tag="rot")
                        nc.vector.tensor_mul(
                            out=t1[:], in0=pq16[:], in1=cos_sb[:, cs:cs + 512])
                        # sin_sb rows are pre-swapped host-side so both SBUF
                        # inputs of each mul share a base partition (walrus
                        # requires it): sin_sb[j] = sin[32+j], sin_sb[32+j]
                        # = -sin[j] within each 64-row head block.
                        for hl in range(2):
                            b0 = 64 * hl
                            nc.vector.tensor_mul(
                                out=rot[b0:b0 + 32, :],
                                in0=pq16[b0 + 32:b0 + 64, :],
                                in1=sin_sb[b0 + 32:b0 + 64, cs:cs + 512])
                            nc.vector.tensor_mul(
                                out=rot[b0 + 32:b0 + 64, :],
                                in0=pq16[b0:b0 + 32, :],
                                in1=sin_sb[b0:b0 + 32, cs:cs + 512])
                        dst = qT if m == 0 else kT
                        nc.vector.tensor_add(
                            out=dst[:, cs:cs + 512], in0=t1[:], in1=rot[:])
                    for tt in range(4):         # v in [token, dim] layout
                        g = ch * 4 + tt
                        pv = pavp.tile([128, 128], F32, tag="pvv")
                        for ko in range(KO):
                            nc.tensor.matmul(
                                pv[:], xc[:, ko, tt * 128:(tt + 1) * 128],
                                wv_sb[:, ko], start=(ko == 0), stop=False)
                        nc.tensor.matmul(
                            pv[:], ones2[:, 0:128], wvb_sb[:],
                            start=False, stop=True)
                        # single strided copy: cols 0:64 -> 0:64,
                        # cols 64:128 -> 65:129 (skips the ones column)
                        nc.scalar.copy(
                            v1[:, g].rearrange("p (u c) -> p u c",
                                               u=2)[:, :, 0:64],
                            pv[:].rearrange("p (u c) -> p u c", u=2))

            # ---------------- Phase B: attention ----------------
            with (
                tc.tile_pool(name="peb", bufs=2) as peb,
                tc.tile_pool(name="pex", bufs=3) as pex,
                tc.tile_pool(name="pem", bufs=3) as pem,
                tc.tile_pool(name="pbn", bufs=2) as pbn,
                tc.tile_pool(name="plp", bufs=2, space="PSUM") as plp,
                tc.tile_pool(name="pvp", bufs=3, space="PSUM") as pvp,
                tc.tile_pool(name="bcp", bufs=1, space="PSUM") as bcp,
            ):
                for qc in range(QC):
                    qs = qc * 512
                    eb = peb.tile([128, 16, 1024], F16, tag="eb",
                                  name=f"eb_{qc}")
                    nc.sync.dma_start(eb[:], expbd[qc])
                    for b in range(2):
                        qtok = b * S + qs
                        pvt = [pvp.tile([65, 512], F32, tag="pv",
                                        name=f"pv_{qc}_{b}_{hl}")
                               for hl in range(2)]
                        for kt in range(16):
                            ktok = b * S + kt * 128
                            pl = plp.tile([128, 1024], F32, tag="pl")
                            for hl in range(2):
                                h0 = 64 * hl
                                nc.tensor.matmul(
                                    pl[:, 512 * hl:512 * hl + 512],
                                    kT[h0:h0 + 64, ktok:ktok + 128],
                                    qT[h0:h0 + 64, qtok:qtok + 512],
                                    start=True, stop=True)
                            ex = pex.tile([128, 1024], F16, tag="ex")
                            nc.scalar.activation(ex[:], pl[:], AF.Exp)
                            exm = pem.tile([128, 1024], F16, tag="exm")
                            nc.vector.tensor_mul(
                                out=exm[:], in0=ex[:], in1=eb[:, kt])
                            for hl in range(2):
                                nc.tensor.matmul(
                                    pvt[hl][:],
                                    v1[:, b * 16 + kt,
                                       65 * hl:65 * hl + 65],
                                    exm[:, 512 * hl:512 * hl + 512],
                                    start=(kt == 0), stop=(kt == 15),
                                    skip_group_check=True)
                        for hl in range(2):
                            h0 = 64 * hl
                            rec2 = pbn.tile([2, 512], F16, tag="rec2")
                            # row 1 must be 0: it pairs with garbage rows
                            nc.vector.memset(rec2[:], 0.0)
                            with nc.allow_low_precision(
                                    reason="1/denom in fp16 (~3e-4 rel)"):
                                nc.vector.reciprocal(rec2[0:1, :],
                                                     pvt[hl][64:65, :])
                            bc = bcp.tile([64, 512], F32, tag="bc")
                            nc.tensor.matmul(bc[:], ones2x64[:], rec2[:],
                                             start=True, stop=True)
                            bcs = pbn.tile([64, 512], F16, tag="bcs")
                            nc.scalar.copy(bcs[:], bc[:])
                            nc.vector.tensor_mul(
                                out=valsT[h0:h0 + 64, qtok:qtok + 512],
                                in0=pvt[hl][0:64, :], in1=bcs[:])

            # ---------------- Phase C: partial o_proj ----------------
            with (
                tc.tile_pool(name="pc", bufs=4) as pc,
                tc.tile_pool(name="pcp", bufs=3, space="PSUM") as pcp,
            ):
                for mt in range(T // 128):
                    for n2 in range(2):
                        po = pcp.tile([128, 512], F32, tag="po")
                        nc.tensor.matmul(
                            po[:], valsT[:, mt * 128:(mt + 1) * 128],
                            wo_sb[:, n2 * 512:(n2 + 1) * 512],
                            start=True, stop=True)
                        ob = pc.tile([128, 512], F16, tag="ob",
                                     name=f"ob_{mt}_{n2}")
                        if (mt * 2 + n2) % 2 == 0:
                            nc.vector.tensor_copy(out=ob[:], in_=po[:])
                        else:
                            nc.scalar.copy(ob[:], po[:])
                        nc.sync.dma_start(
                            outp[mt * 128:(mt + 1) * 128,
                                 n2 * 512:(n2 + 1) * 512], ob[:])

    _split_waits(nc)
    return nc


_nc_cache = None


def _get_nc():
    global _nc_cache
    if _nc_cache is None:
        _nc_cache = _build()
    return _nc_cache


def _prep_inputs(x, pos_bias, sinusoidal_pos, mask, W_qkv, b_qkv, W_o, b_o):
    """Build the 8 per-core input maps (all host-side layout prep)."""
    x = np.asarray(x, np.float32)
    pos_bias = np.asarray(pos_bias, np.float32)
    sp = np.asarray(sinusoidal_pos, np.float32)[0, 0]        # [S, HD]
    mask = np.asarray(mask)
    W_qkv = np.asarray(W_qkv, np.float32)
    b_qkv = np.asarray(b_qkv, np.float32)
    W_o = np.asarray(W_o, np.float32)

    scale = np.float32(1.0 / np.sqrt(HD))

    xflat = x.reshape(T, D)
    # [p, ko, t] -> [p, ch, ko, 512]
    xT_np = np.ascontiguousarray(
        xflat.T.reshape(KO, 128, NCH, 512).transpose(1, 2, 0, 3)
    ).astype(np.float16)

    cos_np = np.cos(sp).T.astype(np.float16)                  # [HD, S]
    sin_t = np.sin(sp).T.astype(np.float32)
    # block-swapped: rows 0:32 hold +sin[32:64] (used for out rows 32:64),
    # rows 32:64 hold -sin[0:32] (used for out rows 0:32)
    sin_np = np.concatenate([sin_t[HD // 2:], -sin_t[:HD // 2]],
                            axis=0).astype(np.float16)

    maskT = (mask[0, 0].T != 0).astype(np.float32)            # [S(k), S(q)]

    # per-head W rows: feature f = h*192 + j (j<64 q, <128 k, <192 v)
    Wh = W_qkv.reshape(H, 3 * HD, D)
    bh = b_qkv.reshape(H, 3 * HD)

    in_maps = []
    for c in range(NCORES):
        h0, h1 = 2 * c, 2 * c + 1
        # q rows scaled by 1/sqrt(HD); k rows unscaled
        Wqk_c = np.concatenate([
            Wh[h0, 0:HD] * scale, Wh[h1, 0:HD] * scale,
            Wh[h0, HD:2 * HD], Wh[h1, HD:2 * HD]], axis=0)    # [256, D]
        bqk_c = np.concatenate([
            bh[h0, 0:HD] * scale, bh[h1, 0:HD] * scale,
            bh[h0, HD:2 * HD], bh[h1, HD:2 * HD]], axis=0)    # [256]
        Wv_c = np.concatenate([Wh[h0, 2 * HD:], Wh[h1, 2 * HD:]], axis=0)
        bv_c = np.concatenate([bh[h0, 2 * HD:], bh[h1, 2 * HD:]], axis=0)

        wqk_np = np.ascontiguousarray(
            Wqk_c.T.reshape(KO, 128, 256).transpose(1, 0, 2)
        ).astype(np.float16)                                   # [128, KO, 256]
        wv_np = np.ascontiguousarray(
            Wv_c.T.reshape(KO, 128, 128).transpose(1, 0, 2)
        ).astype(np.float16)
        wqkb_np = np.zeros((2, 256), np.float16)
        wqkb_np[0] = bqk_c.astype(np.float16)
        wvb_np = np.zeros((2, 128), np.float16)
        wvb_np[0] = bv_c.astype(np.float16)
        wo_np = np.ascontiguousarray(
            W_o[:, 128 * c:128 * (c + 1)].T).astype(np.float16)  # [128, D]

        # expb[qc, kp, kt, hl*512 + qq]
        ebf = np.empty((QC, 128, 16, 1024), np.float16)
        for hl in range(2):
            e = np.exp(pos_bias[0, 2 * c + hl].T * scale) * maskT  # [k, q]
            # [kt, kp, qc, qq] -> [qc, kp, kt, qq]
            ebf[:, :, :, 512 * hl:512 * hl + 512] = (
                e.reshape(16, 128, QC, 512).transpose(2, 1, 0, 3))
        in_maps.append({
            "xT": xT_np, "wqk": wqk_np, "wqkb": wqkb_np,
            "wv": wv_np, "wvb": wvb_np, "wo": wo_np,
            "cosd": cos_np, "sind": sin_np, "expb": ebf,
        })
    return in_maps


def _ensure_profile_hook():
    """Register the axon NTFF profiling hook if the image lacks
    antenv.axon_hooks (needed only for TRACE=True runs)."""
    import sys
    import types
    try:
        from antenv.axon_hooks import get_axon_ntff_profile_hook  # noqa
        return
    except ImportError:
        pass
    try:
        from trn_agent_boot.trn_boot import _ntff_profile_via_ctypes
        hook = _ntff_profile_via_ctypes("/opt/axon/libaxon_pjrt.so")
        mod = types.ModuleType("antenv.axon_hooks")
        mod.get_axon_ntff_profile_hook = lambda: hook
        mod.set_axon_ntff_profile_hook = lambda h: None
        sys.modules["antenv.axon_hooks"] = mod
    except Exception:
        pass


def kernel(x, pos_bias, sinusoidal_pos, mask, W_qkv, b_qkv, W_o, b_o):
    global LAST_RESULT
    if TRACE:
        _ensure_profile_hook()
    in_maps = _prep_inputs(x, pos_bias, sinusoidal_pos, mask,
                           W_qkv, b_qkv, W_o, b_o)
    nc = _get_nc()
    try:
        r = run_bass_kernel_spmd(nc, in_maps, list(range(NCORES)),
                                 trace=TRACE)
    except Exception:
        # occasional transient NRT device errors — retry once
        r = run_bass_kernel_spmd(nc, in_maps, list(range(NCORES)),
                                 trace=TRACE)
    LAST_RESULT = r
    acc = np.zeros((T, D), np.float32)
    for c in range(NCORES):
        acc += r.results[c]["out"].astype(np.float32)
    out = (acc + np.asarray(b_o, np.float32)).astype(np.float32)
    return out.reshape(B, S, D)
